# revision 5
# baseline (speedup 1.0000x reference)
"""Additive attention (Bahdanau) Trainium2 kernel, SPMD over 8 NeuronCores.

Reference computation (per batch b):
    q = queries @ W_q                    [Q, H]
    k = keys    @ W_k                    [K, H]
    scores[q,k] = sum_h w_v[h] * tanh(q[q,h] + k[k,h])
    attn = masked_softmax(scores, valid_len)   (keys >= valid_len masked out)
    out = attn @ values                  [Q, Dv]

Distribution: queries are sharded across the 8 cores (each core gets a
32-query slice of every batch); keys/values/weights are replicated. Each
core computes the same-shaped, perfectly load-balanced slice of the
output. Keys are truncated per batch to the valid length (rounded up to
an even count), which prunes the dominant tanh work for every core
equally. No collectives needed.

Device algorithm per (core, batch):
  - kprojT [h=128, n] = W_k^T @ keys^T      (bf16 weights/data, one DMA
    per batch: kT interleaved as [128, 2, KT] on host)
  - qprojT [h=128, 32] = W_q^T @ q-slice^T
  - x = kprojT + qprojT[:, qi]   (tensor_scalar add, bf16, DVE 2x mode)
  - f = tanh(x)                  (ScalarE, fused over 8-16 queries per op)
  - scoresT[k, qi] = f_chunk^T @ w_v   (PE, N=1 matmuls into PSUM columns)
  - pT = exp(scoresT [+ mask bias on the boundary chunk])
        no max-subtraction needed: |scores| <= ||w_v||_1 ~ 9, exp can't
        overflow fp32; masked lanes get bias -30000 -> exp == 0 exactly.
  - out[qi, v] = (pT^T @ [V | 1]) , then divide by the appended ones-column
    row-sum (softmax normalization). V packed per 128-key chunk on host
    as [128, kcb*130] so each batch's values land in one DMA.
"""

import numpy as np
import ml_dtypes

import concourse.bass as bass
import concourse.tile as tile
import concourse.bacc as bacc
from concourse import mybir
from concourse.bass_utils import run_bass_kernel_spmd

BF16 = mybir.dt.bfloat16
F32 = mybir.dt.float32
TANH = mybir.ActivationFunctionType.Tanh
EXP = mybir.ActivationFunctionType.Exp

B, Q, K, D, H, DV = 8, 256, 1024, 256, 128, 128
NCORES = 8
QSH = Q // NCORES          # queries per core per batch
GQ = 16                    # queries fused per tanh op
VC = DV + 2                # value columns per key chunk (128 vals + ones + pad)
NEG_BIAS = -30000.0        # exp(-30000) == 0.0 exactly in fp32

_graph_cache: dict = {}


def _npad(vl):
    """Per-batch key extent: valid length rounded up to even (DVE 4x mode)."""
    return int(min(max(2 * ((vl + 1) // 2), 2), K))


def _build(nps):
    """Build the SPMD graph. nps: tuple of per-batch (even) key extents."""
    nc = bacc.Bacc("TRN2", target_bir_lowering=False, debug=False,
                   num_devices=NCORES)
    KT = sum(nps)
    kcs = [(n + 127) // 128 for n in nps]
    CK = sum(kcs)
    # batch emission order: smallest first (fast pipeline start), second
    # smallest last (short epilogue tail), the rest largest-first between.
    asc = sorted(range(B), key=lambda b: kcs[b])
    order = [asc[0]] + sorted(asc[2:], key=lambda b: -kcs[b]) + [asc[1]]

    kT_d = nc.dram_tensor("kT", (128, 2, KT), BF16, kind="ExternalInput").ap()
    v_d = nc.dram_tensor("vals", (128, CK * VC), BF16,
                         kind="ExternalInput").ap()
    qT_d = nc.dram_tensor("qT", (128, 2, B * QSH), BF16,
                          kind="ExternalInput").ap()
    # wq halves | wk halves | wv  all bf16, one DMA
    ww_d = nc.dram_tensor("ww", (128, 4 * H + 1), BF16,
                          kind="ExternalInput").ap()
    bias_d = nc.dram_tensor("biasT", (128, B), F32, kind="ExternalInput").ap()
    out_d = nc.dram_tensor("out", (B, QSH, DV), F32, kind="ExternalOutput").ap()

    offs = np.concatenate([[0], np.cumsum(nps)]).astype(int)
    vcols = np.concatenate([[0], np.cumsum([kc * VC for kc in kcs])]).astype(int)

    with tile.TileContext(nc) as tc:
        with (
            tc.tile_pool(name="const", bufs=1) as const,
            tc.tile_pool(name="kt", bufs=5) as kt_pool,
            tc.tile_pool(name="kproj", bufs=4) as kproj_pool,
            tc.tile_pool(name="qproj", bufs=4) as qproj_pool,
            tc.tile_pool(name="vt", bufs=7) as vpool,
            tc.tile_pool(name="x", bufs=4) as xpool,
            tc.tile_pool(name="pT", bufs=2) as ppool,
            tc.tile_pool(name="osb", bufs=2) as osb_pool,
            tc.tile_pool(name="proj_ps", bufs=3, space="PSUM") as proj_ps,
            tc.tile_pool(name="sc_ps", bufs=2, space="PSUM") as sc_ps_pool,
            tc.tile_pool(name="out_ps", bufs=3, space="PSUM") as out_ps_pool,
        ):
            b0 = order[0]
            dmad = {}

            def dma_a(b, first=False):
                n, off, kcb = nps[b], offs[b], kcs[b]
                eng = nc.gpsimd if first else nc.sync
                kt = kt_pool.tile([128, 2, n], BF16, tag="kt")
                eng.dma_start(kt[:], kT_d[:, :, off:off + n])
                qt = kt_pool.tile([128, 2, QSH], BF16, tag="qt")
                eng.dma_start(qt[:], qT_d[:, :, b * QSH:(b + 1) * QSH])
                vt = vpool.tile([128, kcb * VC], BF16, tag="vt")
                nc.gpsimd.dma_start(vt[:], v_d[:, vcols[b]:vcols[b + 1]])
                dmad[b] = (kt, qt, vt)

            # constants first on the sync queue so the projections can start
            # the moment the first batch's kT lands (issued on gpsimd).
            ww_sb = const.tile([128, 4 * H + 1], BF16, tag="ww")
            nc.sync.dma_start(ww_sb[:], ww_d[:, :])
            bias_sb = const.tile([128, B], F32, tag="bias")
            nc.sync.dma_start(bias_sb[:], bias_d[:, :])
            dma_a(b0, first=True)

            wv_sb = ww_sb[:, 4 * H:4 * H + 1]
            projd = {}

            def proj_a(b):
                n = nps[b]
                kt, qt, vt = dmad.pop(b)
                kp = kproj_pool.tile([128, n], BF16, tag="kp")
                for j0 in range(0, n, 512):
                    w = min(512, n - j0)
                    ps = proj_ps.tile([128, w], F32, tag="ps")
                    nc.tensor.matmul(ps[:], ww_sb[:, 2 * H:3 * H],
                                     kt[:, 0, j0:j0 + w], start=True,
                                     stop=False)
                    nc.tensor.matmul(ps[:], ww_sb[:, 3 * H:4 * H],
                                     kt[:, 1, j0:j0 + w], start=False,
                                     stop=True)
                    nc.vector.tensor_copy(kp[:, j0:j0 + w], ps[:])
                qp = qproj_pool.tile([128, QSH], F32, tag="qp")
                ps = proj_ps.tile([128, QSH], F32, tag="ps")
                nc.tensor.matmul(ps[:], ww_sb[:, 0:H], qt[:, 0, :],
                                 start=True, stop=False)
                nc.tensor.matmul(ps[:], ww_sb[:, H:2 * H], qt[:, 1, :],
                                 start=False, stop=True)
                nc.vector.tensor_copy(qp[:], ps[:])
                projd[b] = (kp, qp, vt)

            for bb in order[1:4]:
                dma_a(bb)
            proj_a(b0)

            # ---- per-batch pipeline ----
            # DMAs issued 3 batches ahead; projections 1 batch ahead;
            # exp+final-matmul of batch i-1 fire after batch i's first
            # score group (g==1 slot: extra PE slack); divides of batch
            # i-2 at g==0.
            pend_exp = None
            div_q = []
            for bi, b in enumerate(order):
                n = nps[b]
                kcb = kcs[b]
                m_last = n - (kcb - 1) * 128
                kp_b, qp_b, vt_b = projd.pop(b)
                sc = sc_ps_pool.tile([128, kcb * QSH], F32, tag="sc")
                if m_last < 128:
                    # kill stale PSUM rows in the partial chunk so
                    # exp(stale + bias) can't produce inf/nan; partition
                    # base must be 32-aligned, matmuls rewrite [0,m_last)
                    m0 = (m_last // 32) * 32
                    for p0 in range(m0, 128, 32):
                        nc.vector.memset(sc[p0:p0 + 32, (kcb - 1) * QSH:], 0.0)
                for g in range(QSH // GQ):
                    x = xpool.tile([128, GQ * n], BF16, tag="x")
                    for j in range(GQ):
                        qi = g * GQ + j
                        nc.vector.tensor_scalar_add(
                            x[:, j * n:(j + 1) * n], kp_b[:],
                            qp_b[:, qi:qi + 1])
                    if bi == 0 and g == 0:
                        # two half-ops so the very first tanh starts after
                        # 8 adds instead of 16 (trims pipeline startup)
                        h = (GQ // 2) * n
                        nc.scalar.activation(x[:, 0:h], x[:, 0:h], TANH)
                        nc.scalar.activation(x[:, h:], x[:, h:], TANH)
                    else:
                        nc.scalar.activation(x[:], x[:], TANH)  # in-place
                    for j in range(GQ):
                        qi = g * GQ + j
                        for c in range(kcb):
                            m = min(128, n - c * 128)
                            col = c * QSH + qi
                            nc.tensor.matmul(
                                sc[:m, col:col + 1],
                                x[:, j * n + c * 128:j * n + c * 128 + m],
                                wv_sb,
                                start=True, stop=True)
                    if g == 0:
                        if div_q and len(div_q) >= 2:
                            div_q.pop(0)()
                        if bi + 4 < B:
                            dma_a(order[bi + 4])
                        if bi + 1 < B:
                            proj_a(order[bi + 1])
                    else:
                        if pend_exp is not None:
                            pend_exp()
                            pend_exp = None
                state = {}

                def make_exp_final(b=b, kcb=kcb, sc=sc, vt_b=vt_b,
                                   state=state):
                    def exp_final():
                        pT = ppool.tile([128, kcb * QSH], BF16, tag="pT")
                        last0 = (kcb - 1) * QSH
                        if kcb > 1:
                            nc.scalar.activation(pT[:, 0:last0],
                                                 sc[:, 0:last0], EXP)
                        nc.scalar.activation(pT[:, last0:], sc[:, last0:],
                                             EXP, bias=bias_sb[:, b:b + 1])
                        ops = out_ps_pool.tile([QSH, DV + 1], F32, tag="ops")
                        for c in range(kcb):
                            nc.tensor.matmul(
                                ops[:],
                                pT[:, c * QSH:(c + 1) * QSH],
                                vt_b[:, c * VC:c * VC + DV + 1],
                                start=(c == 0), stop=(c == kcb - 1))
                        state["ops"] = ops
                    return exp_final

                def make_div(b=b, state=state):
                    def div():
                        ops = state["ops"]
                        r = osb_pool.tile([QSH, 1], F32, tag="r")
                        nc.vector.reciprocal(r[:], ops[:, DV:DV + 1])
                        osb = osb_pool.tile([QSH, DV], F32, tag="osb")
                        nc.vector.tensor_scalar_mul(osb[:], ops[:, 0:DV], r[:])
                        nc.sync.dma_start(out_d[b, :, :], osb[:])
                    return div

                pend_exp = make_exp_final()
                div_q.append(make_div())
            pend_exp()
            for dv in div_q:
                dv()
    nc.compile()
    return nc


def _prep(queries, keys, values, valid_lens):
    vl = np.asarray(valid_lens).astype(np.int64)
    nps = tuple(_npad(int(l)) for l in vl)
    KT = sum(nps)
    kcs = [(n + 127) // 128 for n in nps]
    CK = sum(kcs)

    kT = np.zeros((128, 2, KT), ml_dtypes.bfloat16)
    vals = np.zeros((128, CK * VC), ml_dtypes.bfloat16)
    biasT = np.zeros((128, B), np.float32)
    off = 0
    vcol = 0
    for b in range(B):
        n = nps[b]
        keysT = keys[b, :n, :].T.astype(ml_dtypes.bfloat16)  # [D, n]
        kT[:, 0, off:off + n] = keysT[0:128, :]
        kT[:, 1, off:off + n] = keysT[128:256, :]
        kcb = kcs[b]
        for c in range(kcb):
            mc = min(128, n - c * 128)
            vals[0:mc, vcol + c * VC:vcol + c * VC + DV] = (
                values[b, c * 128:c * 128 + mc, :].astype(ml_dtypes.bfloat16))
            vals[0:mc, vcol + c * VC + DV] = ml_dtypes.bfloat16(1.0)
        j = np.arange(128)
        valid = (kcb - 1) * 128 + j < vl[b]
        biasT[:, b] = np.where(valid, 0.0, NEG_BIAS).astype(np.float32)
        off += n
        vcol += kcb * VC

    qT_shards = []
    for i in range(NCORES):
        qt = np.zeros((128, 2, B * QSH), ml_dtypes.bfloat16)
        for b in range(B):
            qsT = queries[b, i * QSH:(i + 1) * QSH, :].T.astype(
                ml_dtypes.bfloat16)  # [D, QSH]
            qt[:, 0, b * QSH:(b + 1) * QSH] = qsT[0:128, :]
            qt[:, 1, b * QSH:(b + 1) * QSH] = qsT[128:256, :]
        qT_shards.append(qt)
    return nps, kT, vals, biasT, qT_shards


def run(queries, keys, values, valid_lens, W_q, W_k, w_v, **run_kwargs):
    """Full pipeline; returns (output, BassKernelResults)."""
    queries = np.asarray(queries, np.float32)
    keys = np.asarray(keys, np.float32)
    values = np.asarray(values, np.float32)
    W_q = np.asarray(W_q, np.float32)
    W_k = np.asarray(W_k, np.float32)
    w_v = np.asarray(w_v, np.float32)

    nps, kT, vals, biasT, qT_shards = _prep(queries, keys, values, valid_lens)
    ww = np.zeros((128, 4 * H + 1), ml_dtypes.bfloat16)
    ww[:, 0:H] = W_q[0:128, :].astype(ml_dtypes.bfloat16)
    ww[:, H:2 * H] = W_q[128:256, :].astype(ml_dtypes.bfloat16)
    ww[:, 2 * H:3 * H] = W_k[0:128, :].astype(ml_dtypes.bfloat16)
    ww[:, 3 * H:4 * H] = W_k[128:256, :].astype(ml_dtypes.bfloat16)
    ww[:, 4 * H] = w_v.astype(ml_dtypes.bfloat16)
    common = {
        "kT": np.ascontiguousarray(kT),
        "vals": np.ascontiguousarray(vals),
        "ww": np.ascontiguousarray(ww),
        "biasT": np.ascontiguousarray(biasT),
    }
    in_maps = [dict(common, qT=np.ascontiguousarray(q)) for q in qT_shards]

    nc = _graph_cache.get(nps)
    if nc is None:
        nc = _build(nps)
        _graph_cache[nps] = nc
    res = run_bass_kernel_spmd(nc, in_maps, core_ids=list(range(NCORES)),
                               **run_kwargs)
    out = np.empty((B, Q, DV), np.float32)
    for i in range(NCORES):
        out[:, i * QSH:(i + 1) * QSH, :] = res.results[i]["out"]
    return out, res


def kernel(queries, keys, values, valid_lens, W_q, W_k, w_v):
    out, _ = run(queries, keys, values, valid_lens, W_q, W_k, w_v)
    return out
